# revision 30
# baseline (speedup 1.0000x reference)
"""Two-layer GAT (graph attention) kernel for 8 Trainium2 NeuronCores.

v2 strategy (destination-sharded edge parallelism, gather-prep optimized):
  * Nodes are degree-sorted and dealt round-robin to the 8 cores; each core
    aggregates messages for its own 6250 nodes only (no cross-core reduce).
  * Sharded front end: each core computes the layer-1 fat-row table for ITS
    stripe only (bf16, 512B rows: [h0|1|h1|1|h2|1|h3|1|a_src(4)|pad]), then an
    AllGather replicates the full table to every core's HBM.  The interleaved
    "ones" columns make the attention denominator fall out of the same
    slot-reduce as the messages (no separate denominator reduce).
  * Per-edge rows are fetched with dma_gather (SWDGE).  The Pool-engine
    descriptor-prep cost is linear in the static index count, so padding is
    minimized with OVERLAPPED index tables: table A = rows [0, 32768),
    table B = rows [17408, 50176) of the same tensor (int16 index range fits
    both exactly).  Edges whose source falls in the overlap are assigned to
    whichever side balances that destination's A/B slot counts.
  * A and B gathers of two consecutive destination blocks land in ONE SBUF
    tile (4 blocks worth for layer 2), halving per-gather fixed costs.
    Gathers rotate over 4 SWDGE queues (4 Q7 cpu pairs, 4 descriptor rings).
  * Padding slots point at a dummy row whose alpha is -1e30 => exp() == 0.
  * Layer 2 repeats the scheme with 256B f32 rows [h2(32)|1|a2_src|pad].

The host side (pure numpy) permutes nodes, builds the padded gather index
lists, and un-permutes the result.
"""

import sys

sys.path.insert(0, "/opt/trn_rl_repo")

import numpy as np

import concourse.bacc as bacc
import concourse.bass as bass
import concourse.mybir as mybir
import concourse.tile as tile
from concourse.bass_utils import run_bass_kernel_spmd

F32 = mybir.dt.float32
BF16 = mybir.dt.bfloat16
I16 = mybir.dt.int16
AL = mybir.AluOpType
ACT = mybir.ActivationFunctionType

CORES = 8
NEG_SLOPE = 0.2
NEG_BIG = -1.0e30

# problem constants (nn_GAT_35296041238878)
N = 50000
IN_DIM = 128
HID = 32
HEADS = 4
OUT_DIM = 32

# layer-1 fat row (bf16): [h0(32)|1|h1(32)|1|h2(32)|1|h3(32)|1|asrc(4)|pad] = 256
L1_ROW = 256
L1_USE = HEADS * (HID + 1)          # 132 (h+ones)
L1H = HEADS * HID                   # 128
W1N = L1H + 2 * HEADS               # 136 matmul cols [h|asrc|adst]
# layer-2 fat row (f32): [h2(32)|1|a2s|pad] = 64
L2_ROW = 64
L2_USE = OUT_DIM + 1                # 33
W2N = OUT_DIM + 2                   # 34 matmul cols [h2|a2s|a2d]

NQ = 4                              # SWDGE queues
L1_GRP = 2                          # dst blocks per gather, layer 1
L2_GRP = 4                          # dst blocks per gather, layer 2

_CACHE = {}

# ---------------------------------------------------------------------------
# Tile's DMASW lane round-robin is not SWDGE-queue-aware: a lane semaphore is
# locked to the queue of its first user, so rotating queue_num with the
# default assignment trips "locked to SWDGE queue" at schedule time.
# Partition the 8 lanes: queue q -> lanes [q*2, q*2+2).
import concourse.tile_sem_assignment as _tsa


def _queue_aware_assign_tick(self, inst):
    q = getattr(inst, "queue_num", None)
    if q is not None and isinstance(inst, _tsa.DMAInst) \
            and inst.engine == _tsa.mybir.EngineType.Pool:
        if not hasattr(self, "_q_lane_ctr"):
            self._q_lane_ctr = {}
        ctr = self._q_lane_ctr.get(q, 0)
        self._q_lane_ctr[q] = ctr + 1
        lanes = max(1, self.swdge_sem_count // NQ)
        self.next_sw_dma_idx = (q % NQ) * lanes + (ctr % lanes)
    return _tsa.TileClockTick._orig_assign_tick(self, inst)


if not hasattr(_tsa.TileClockTick, "_orig_assign_tick"):
    _tsa.TileClockTick._orig_assign_tick = _tsa.TileClockTick._assign_tick
    _tsa.TileClockTick._assign_tick = _queue_aware_assign_tick


# ----------------------------------------------------------------------------
# host-side graph preprocessing
# ----------------------------------------------------------------------------
def _prep_graph(edge_index, n_nodes, bpc):
    """Permute nodes, shard by destination, build padded gather index lists.

    Index tables overlap: A = rows [0, 32768), B = rows [BBASE, tbl_rows)
    with BBASE = tbl_rows - 32768.  Edges with src pos in the overlap are
    assigned to balance each node's A/B slot counts.
    """
    npc = n_nodes // CORES           # real nodes per core
    stride = bpc * 128               # table stripe per core (rows >= npc: dummy)
    tbl_rows = CORES * stride
    bbase = tbl_rows - 32768
    assert npc < stride and bbase >= 0 and tbl_rows - bbase == 32768
    a_dummy = npc                    # core-0 stripe dummy row, < 32768
    bd_core = next(c for c in range(CORES) if c * stride + npc >= bbase)
    b_dummy_local = bd_core * stride + npc - bbase
    assert 0 <= b_dummy_local < 32768

    src = np.concatenate([edge_index[0], np.arange(n_nodes)]).astype(np.int64)
    dst = np.concatenate([edge_index[1], np.arange(n_nodes)]).astype(np.int64)

    deg = np.bincount(dst, minlength=n_nodes)
    order = np.argsort(-deg, kind="stable")
    # rank r -> core r%8, local row r//8  (degree-balanced, within-core sorted)
    pos = np.empty(n_nodes, dtype=np.int64)
    ranks = np.arange(n_nodes)
    pos[order] = (ranks % CORES) * stride + ranks // CORES
    nodes_of_core = [order[c::CORES] for c in range(CORES)]

    dpos = pos[dst]
    e_core = dpos // stride
    ld = dpos % stride               # local dst row, < npc
    sp = pos[src]                    # source table position

    # ---- balanced A/B side assignment ----
    key = e_core * stride + ld       # destination node's table row
    fixedB = sp >= 32768
    flex = (sp >= bbase) & ~fixedB
    degn = np.bincount(key, minlength=tbl_rows)
    nA_fixed = np.bincount(key[sp < bbase], minlength=tbl_rows)
    nF = np.bincount(key[flex], minlength=tbl_rows)
    tgtA = np.minimum(np.maximum((degn + 1) // 2, nA_fixed), nA_fixed + nF)
    # rank of each flex edge within its key
    fidx = np.flatnonzero(flex)
    o = np.argsort(key[fidx], kind="stable")
    fs = fidx[o]
    ks = key[fs]
    change = np.r_[True, ks[1:] != ks[:-1]]
    starts = np.flatnonzero(change)
    gid = np.cumsum(change) - 1
    frank = np.arange(len(fs)) - starts[gid]
    sideB = fixedB.copy()
    sideB[fs] = frank >= (tgtA - nA_fixed)[ks]

    nA = np.bincount(key[~sideB], minlength=tbl_rows)
    nB = degn - nA

    def blockmax(x):
        return x.reshape(CORES, bpc, 128).max(axis=0).max(axis=1)

    da = np.maximum(blockmax(nA), 1)
    db = np.maximum(blockmax(nB), 1)
    offa = np.concatenate([[0], np.cumsum(da)])
    offb = np.concatenate([[0], np.cumsum(db)])

    idxa_list, idxb_list = [], []
    for c in range(CORES):
        m = e_core == c
        ldc, spc, sbc = ld[m], sp[m], sideB[m]
        o2 = np.lexsort((sbc, ldc))
        ldc, spc, sbc = ldc[o2], spc[o2], sbc[o2]
        keyc = ldc * 2 + sbc
        change = np.r_[True, keyc[1:] != keyc[:-1]]
        gid = np.cumsum(change) - 1
        starts = np.flatnonzero(change)
        jj = np.arange(len(ldc)) - starts[gid]
        bidx = ldc // 128
        d = ldc % 128
        flat_a = np.full(128 * offa[-1], a_dummy, dtype=np.int64)
        flat_b = np.full(128 * offb[-1], b_dummy_local, dtype=np.int64)
        ma = ~sbc
        flat_a[(offa[bidx[ma]] + jj[ma]) * 128 + d[ma]] = spc[ma]
        mb = sbc
        flat_b[(offb[bidx[mb]] + jj[mb]) * 128 + d[mb]] = spc[mb] - bbase
        assert flat_a.max() < 32768 and flat_b.max() < 32768
        # wrap per block: i -> [i%16, i//16], concat blocks along columns
        wa = np.concatenate(
            [flat_a[128 * offa[b]:128 * offa[b + 1]].reshape(-1, 16).T
             for b in range(bpc)], axis=1).astype(np.int16)
        wb = np.concatenate(
            [flat_b[128 * offb[b]:128 * offb[b + 1]].reshape(-1, 16).T
             for b in range(bpc)], axis=1).astype(np.int16)
        idxa_list.append(np.tile(wa, (8, 1)))
        idxb_list.append(np.tile(wb, (8, 1)))

    return dict(
        npc=npc, stride=stride, tbl_rows=tbl_rows, bbase=bbase, bpc=bpc,
        da=da.astype(int).tolist(), db=db.astype(int).tolist(),
        offa=offa.astype(int).tolist(), offb=offb.astype(int).tolist(),
        pos=pos, nodes_of_core=nodes_of_core,
        idxa=idxa_list, idxb=idxb_list,
    )


# ----------------------------------------------------------------------------
# device program
# ----------------------------------------------------------------------------
def _build_program(g, has_b1):
    bpc, stride, tbl_rows, bbase = g["bpc"], g["stride"], g["tbl_rows"], g["bbase"]
    da, db, offa, offb = g["da"], g["db"], g["offa"], g["offb"]
    npc = g["npc"]
    sa_cols = 8 * offa[-1]
    sb_cols = 8 * offb[-1]

    nc = bacc.Bacc("TRN2", target_bir_lowering=False, debug=False,
                   num_devices=CORES, num_swdge_queues=NQ)

    xTs = nc.dram_tensor("xTs", [128, stride], BF16, kind="ExternalInput")
    w1e = nc.dram_tensor("w1e", [128, W1N], BF16, kind="ExternalInput")
    w2e = nc.dram_tensor("w2e", [L1H, W2N], BF16, kind="ExternalInput")
    b1t = nc.dram_tensor("b1t", [128, L1H], F32, kind="ExternalInput")
    ident = nc.dram_tensor("ident", [128, 128], F32, kind="ExternalInput")
    idxa = nc.dram_tensor("idxa", [128, sa_cols], I16, kind="ExternalInput")
    idxb = nc.dram_tensor("idxb", [128, sb_cols], I16, kind="ExternalInput")

    cc1 = nc.dram_tensor("cc1", [stride, L1_ROW], BF16)
    tbl1 = nc.dram_tensor("tbl1", [tbl_rows, L1_ROW], BF16, addr_space="Shared")
    cc2 = nc.dram_tensor("cc2", [stride, L2_ROW], F32)
    tbl2 = nc.dram_tensor("tbl2", [tbl_rows, L2_ROW], F32, addr_space="Shared")
    out = nc.dram_tensor("out", [stride, OUT_DIM], F32, kind="ExternalOutput")

    with tile.TileContext(nc) as tc:
        with (
            tc.tile_pool(name="res", bufs=1) as res,
            tc.tile_pool(name="ps", bufs=2, space="PSUM") as psp,
            tc.tile_pool(name="sml", bufs=2) as sml,
        ):
            # ---- resident constants ----
            w1e_t = res.tile([128, W1N], BF16, tag="w1e")
            nc.sync.dma_start(w1e_t[:], w1e.ap())
            w2e_t = res.tile([L1H, W2N], BF16, tag="w2e")
            nc.sync.dma_start(w2e_t[:], w2e.ap())
            b1_t = res.tile([128, L1H], F32, tag="b1")
            nc.sync.dma_start(b1_t[:], b1t.ap())
            id_t = res.tile([128, 128], F32, tag="ident")
            nc.sync.dma_start(id_t[:], ident.ap())
            ia_t = res.tile([128, sa_cols], I16, tag="idxa")
            nc.sync.dma_start(ia_t[:], idxa.ap())
            ib_t = res.tile([128, sb_cols], I16, tag="idxb")
            nc.sync.dma_start(ib_t[:], idxb.ap())
            ad_own = res.tile([128, bpc * HEADS], F32, tag="adown")
            ad2_own = res.tile([128, bpc], F32, tag="ad2own")

            # dummy rows [npc, stride) of both cc tensors: alpha = -1e30
            pad_rows = stride - npc
            dmy1 = res.tile([pad_rows, L1_ROW], BF16, tag="dmy1")
            nc.vector.memset(dmy1[:], 0.0)
            nc.vector.memset(dmy1[:, L1_USE:L1_USE + HEADS], NEG_BIG)
            nc.sync.dma_start(cc1.ap()[npc:stride, :], dmy1[:])
            # l2 row layout: [h2(0:32) | a2s(32) | one(33) | pad]
            dmy2 = res.tile([pad_rows, L2_ROW], F32, tag="dmy2")
            nc.vector.memset(dmy2[:], 0.0)
            nc.vector.memset(dmy2[:, OUT_DIM:OUT_DIM + 1], NEG_BIG)
            nc.sync.dma_start(cc2.ap()[npc:stride, :], dmy2[:])

            # ---- front end: this core's stripe of the fat-row table ----
            fe_ctx = tc.tile_pool(name="fe", bufs=3)
            fe = fe_ctx.__enter__()
            FCH = 4                   # blocks per cc1 write
            # pre-zero the 3 rotating fat buffers once; pads stay zero
            for _ in range(3):
                f0 = fe.tile([128, FCH, L1_ROW], BF16, tag="fat")
                nc.vector.memset(f0[:].rearrange("p a b -> p (a b)"), 0.0)
            tbl1_v = tbl1.ap().rearrange("(c s) e -> c s e", c=CORES)
            CH_ROWS = 1536            # AG chunk rows (12 frontend blocks)
            ag1_fired = 0

            for t0 in range(0, bpc, FCH):
                tn = min(FCH, bpc - t0)
                fat = fe.tile([128, FCH, L1_ROW], BF16, tag="fat")
                for k in range(tn):
                    t = t0 + k
                    xt = fe.tile([128, 128], BF16, tag="xt")
                    nc.sync.dma_start(xt[:], xTs.ap()[:, 128 * t:128 * (t + 1)])
                    ps = psp.tile([128, W1N], F32, tag="feps")
                    nc.tensor.matmul(ps[:], xt[:], w1e_t[:], start=True, stop=True)
                    fk = fat[:, k, :]
                    f4 = fk[:, 0:L1_USE].rearrange("p (h c) -> p h c", h=HEADS)
                    nc.vector.tensor_copy(
                        f4[:, :, 0:HID],
                        ps[:, 0:L1H].rearrange("p (h c) -> p h c", h=HEADS))
                    nc.vector.memset(f4[:, :, HID:HID + 1], 1.0)
                    nc.vector.tensor_copy(
                        fk[:, L1_USE:L1_USE + HEADS], ps[:, L1H:L1H + HEADS])
                    nc.vector.tensor_copy(
                        ad_own[:, HEADS * t:HEADS * (t + 1)],
                        ps[:, L1H + HEADS:L1H + 2 * HEADS])
                nrows = min(128 * tn, npc - 128 * t0)
                if nrows == 128 * tn:
                    nc.sync.dma_start(
                        cc1.ap()[128 * t0:128 * t0 + nrows, :].rearrange(
                            "(t p) e -> p t e", p=128), fat[:, 0:tn, :])
                else:
                    for k in range(tn):
                        t = t0 + k
                        nr = min(128, npc - 128 * t)
                        if nr > 0:
                            nc.sync.dma_start(
                                cc1.ap()[128 * t:128 * t + nr, :],
                                fat[0:nr, k, :])

            fe_ctx.__exit__(None, None, None)
            tc.strict_bb_all_engine_barrier()
            nc.gpsimd.collective_compute(
                "AllGather", AL.bypass,
                replica_groups=[list(range(CORES))],
                ins=[cc1.ap().opt()], outs=[tbl1.ap().opt()])
            tc.strict_bb_all_engine_barrier()

            # ---- layer 1: per-block gathers, tree slot-reduce ----
            l1_gat_ctx = tc.tile_pool(name="gat1", bufs=4)
            gat = l1_gat_ctx.__enter__()
            l1_mid_ctx = tc.tile_pool(name="mid1", bufs=2)
            mid = l1_mid_ctx.__enter__()
            # persistent double-buffered l2fat with constant cols pre-set
            l2f_tiles = []
            for i in range(2):
                lf = res.tile([128, L2_ROW], F32, tag=f"l2f{i}")
                nc.vector.memset(lf[:, OUT_DIM + 2:L2_ROW], 0.0)
                nc.vector.memset(lf[:, OUT_DIM + 1:OUT_DIM + 2], 1.0)
                l2f_tiles.append(lf)
            tblA = tbl1.ap()[0:32768, :]
            tblB = tbl1.ap()[bbase:tbl_rows, :]
            tbl2_v = tbl2.ap().rearrange("(c s) e -> c s e", c=CORES)
            ag2_fired = 0

            def tree_reduce(m, D, W):
                """In-place pairwise slot reduce of m[:, 0:D, 0:W] -> m[:,0,:].

                All adds are on flat contiguous [128, k*W] slabs.
                """
                Dt = 1 << (D.bit_length() - 1)
                if Dt == D and D > 1:
                    Dt >>= 1
                if D > Dt:
                    k = D - Dt
                    nc.vector.tensor_tensor(
                        m[:, 0:k, :].rearrange("p a b -> p (a b)"),
                        m[:, 0:k, :].rearrange("p a b -> p (a b)"),
                        m[:, Dt:D, :].rearrange("p a b -> p (a b)"), AL.add)
                k = Dt >> 1
                while k >= 1:
                    nc.vector.tensor_tensor(
                        m[:, 0:k, :].rearrange("p a b -> p (a b)"),
                        m[:, 0:k, :].rearrange("p a b -> p (a b)"),
                        m[:, k:2 * k, :].rearrange("p a b -> p (a b)"), AL.add)
                    k >>= 1

            def split_gathers(out_tile, tblA_ap, tblB_ap, idx_a, idx_b,
                              b, row, q0):
                """4 gathers per block (A and B halves) on 4 distinct queues."""
                DA, DB = da[b], db[b]
                parts = []
                hA = DA // 2
                if hA >= 1:
                    parts.append((out_tile[:, 0:hA, :], tblA_ap,
                                  idx_a[:, 8 * offa[b]:8 * (offa[b] + hA)], hA))
                    parts.append((out_tile[:, hA:DA, :], tblA_ap,
                                  idx_a[:, 8 * (offa[b] + hA):8 * offa[b + 1]],
                                  DA - hA))
                else:
                    parts.append((out_tile[:, 0:DA, :], tblA_ap,
                                  idx_a[:, 8 * offa[b]:8 * offa[b + 1]], DA))
                hB = DB // 2
                if hB >= 1:
                    parts.append((out_tile[:, DA:DA + hB, :], tblB_ap,
                                  idx_b[:, 8 * offb[b]:8 * (offb[b] + hB)], hB))
                    parts.append((out_tile[:, DA + hB:DA + DB, :], tblB_ap,
                                  idx_b[:, 8 * (offb[b] + hB):8 * offb[b + 1]],
                                  DB - hB))
                else:
                    parts.append((out_tile[:, DA:DA + DB, :], tblB_ap,
                                  idx_b[:, 8 * offb[b]:8 * offb[b + 1]], DB))
                for i, (oap, tap, iap, dn) in enumerate(parts):
                    nc.gpsimd.dma_gather(
                        oap, tap, iap, 128 * dn, 128 * dn, row,
                        elem_step=row, single_packet=False,
                        queue_num=(q0 + i) % NQ)

            for b in range(bpc):
                DA, DB = da[b], db[b]
                D = DA + DB
                gt = gat.tile([128, D, L1_ROW], BF16, tag="g")
                split_gathers(gt, tblA, tblB, ia_t, ib_t, b, L1_ROW, b % NQ)

                adb = ad_own[:, HEADS * b:HEADS * (b + 1)]
                z = sml.tile([128, D, HEADS], F32, tag="z")
                nc.vector.tensor_tensor(
                    z[:, :, :], gt[:, :, L1_USE:L1_USE + HEADS],
                    adb.unsqueeze(1).broadcast_to([128, D, HEADS]), AL.add)
                z2 = sml.tile([128, D, HEADS], F32, tag="z2")
                nc.vector.scalar_tensor_tensor(
                    z2[:].rearrange("p a b -> p (a b)"),
                    z[:].rearrange("p a b -> p (a b)"), NEG_SLOPE,
                    z[:].rearrange("p a b -> p (a b)"),
                    op0=AL.mult, op1=AL.max)
                wb = sml.tile([128, D, HEADS], BF16, tag="wb")
                nc.scalar.activation(
                    wb[:].rearrange("p a b -> p (a b)"),
                    z2[:].rearrange("p a b -> p (a b)"), ACT.Exp)

                m = mid.tile([128, D, L1_USE], F32, tag="m")
                m4 = m[:, :, :].rearrange("p d (h c) -> p d h c", h=HEADS)
                nc.vector.tensor_tensor(
                    m4, gt[:, :, 0:L1_USE].rearrange(
                        "p d (h c) -> p d h c", h=HEADS),
                    wb[:, :, :].unsqueeze(3).broadcast_to(
                        [128, D, HEADS, HID + 1]), AL.mult)
                tree_reduce(m, D, L1_USE)
                r4 = m[:, 0, :].rearrange("p (h c) -> p h c", h=HEADS)

                rec = sml.tile([128, HEADS], F32, tag="rec")
                nc.vector.reciprocal(rec[:], r4[:, :, HID])
                o1 = sml.tile([128, L1H], F32, tag="o1")
                nc.vector.tensor_tensor(
                    o1[:].rearrange("p (h c) -> p h c", h=HEADS),
                    r4[:, :, 0:HID],
                    rec[:].unsqueeze(2).broadcast_to([128, HEADS, HID]),
                    AL.mult)
                if has_b1:
                    o1b = sml.tile([128, L1H], F32, tag="o1b")
                    nc.vector.tensor_tensor(o1b[:], o1[:], b1_t[:, :], AL.add)
                else:
                    o1b = o1
                # elu(x) = max(x, exp(min(x,0)) - 1);  e1n = -min(x,0) = relu(-x)
                e1n = sml.tile([128, L1H], F32, tag="e1n")
                nc.scalar.activation(e1n[:], o1b[:], ACT.Relu, scale=-1.0)
                e2 = sml.tile([128, L1H], F32, tag="e2")
                nc.scalar.activation(e2[:], e1n[:], ACT.Exp, scale=-1.0)
                elu = sml.tile([128, L1H], F32, tag="elu")
                nc.vector.scalar_tensor_tensor(
                    elu[:], e2[:], -1.0, o1b[:], op0=AL.add, op1=AL.max)
                # h2' = elu^T @ W2ext
                tp = psp.tile([128, 128], F32, tag="tp")
                nc.tensor.transpose(tp[:], elu[:], id_t[:])
                eluT = sml.tile([128, 128], BF16, tag="eluT")
                nc.scalar.activation(eluT[:], tp[:], ACT.Copy)
                h2p = psp.tile([128, W2N], F32, tag="h2p")
                nc.tensor.matmul(h2p[:], eluT[:], w2e_t[:],
                                 start=True, stop=True)
                l2fat = l2f_tiles[b % 2]
                # l2fat row: [h2(0:32) | a2s(32) | one(33) | pad]
                nc.scalar.activation(
                    l2fat[:, 0:OUT_DIM + 1], h2p[:, 0:OUT_DIM + 1], ACT.Copy)
                nc.scalar.activation(
                    ad2_own[:, b:b + 1], h2p[:, OUT_DIM + 1:OUT_DIM + 2],
                    ACT.Copy)
                nrows = min(128, npc - 128 * b)
                nc.sync.dma_start(
                    cc2.ap()[128 * b:128 * b + nrows, :], l2fat[0:nrows, :])

            l1_mid_ctx.__exit__(None, None, None)
            l1_gat_ctx.__exit__(None, None, None)
            tc.strict_bb_all_engine_barrier()
            nc.gpsimd.collective_compute(
                "AllGather", AL.bypass,
                replica_groups=[list(range(CORES))],
                ins=[cc2.ap().opt()], outs=[tbl2.ap().opt()])
            tc.strict_bb_all_engine_barrier()

            # ---- layer 2: per-block gathers, tree slot-reduce ----
            l2_gat_ctx = tc.tile_pool(name="gat2", bufs=4)
            gat = l2_gat_ctx.__enter__()
            l2_mid_ctx = tc.tile_pool(name="mid2", bufs=2)
            mid = l2_mid_ctx.__enter__()
            t2A = tbl2.ap()[0:32768, :]
            t2B = tbl2.ap()[bbase:tbl_rows, :]
            W2R = OUT_DIM + 2      # reduce width: [h2|a2s(junk)|one]
            for b in range(bpc):
                DA, DB = da[b], db[b]
                D = DA + DB
                g2 = gat.tile([128, D, L2_ROW], F32, tag="g2")
                split_gathers(g2, t2A, t2B, ia_t, ib_t, b, L2_ROW, b % NQ)

                # z = a2s[src] + a2d[dst] on the scalar engine (strided read)
                z = sml.tile([128, D], F32, tag="z2l")
                nc.scalar.activation(
                    z[:, :], g2[:, :, OUT_DIM], ACT.Identity,
                    bias=ad2_own[:, b:b + 1])
                z2 = sml.tile([128, D], F32, tag="z2l2")
                nc.vector.scalar_tensor_tensor(
                    z2[:, :], z[:, :], NEG_SLOPE, z[:, :],
                    op0=AL.mult, op1=AL.max)
                w2t = sml.tile([128, D], F32, tag="w2t")
                nc.scalar.activation(w2t[:, :], z2[:, :], ACT.Exp)

                m2 = mid.tile([128, D, W2R], F32, tag="m2")
                nc.vector.tensor_tensor(
                    m2[:, :, :], g2[:, :, 0:W2R],
                    w2t[:, :].unsqueeze(2).broadcast_to([128, D, W2R]),
                    AL.mult)
                tree_reduce(m2, D, W2R)
                r = m2[:, 0, :]

                rec = sml.tile([128, 1], F32, tag="rec2")
                nc.vector.reciprocal(rec[:], r[:, OUT_DIM + 1:OUT_DIM + 2])
                o2 = sml.tile([128, OUT_DIM], F32, tag="o2")
                nc.vector.tensor_scalar(
                    o2[:], r[:, 0:OUT_DIM], rec[:], None, op0=AL.mult)
                nrows = min(128, npc - 128 * b)
                nc.sync.dma_start(
                    out.ap()[128 * b:128 * b + nrows, :], o2[0:nrows, :])

            l2_mid_ctx.__exit__(None, None, None)
            l2_gat_ctx.__exit__(None, None, None)

    nc.compile()
    return nc


# ----------------------------------------------------------------------------
# weight prep + end-to-end run
# ----------------------------------------------------------------------------
def _run(x, edge_index, W1, a1_src, a1_dst, b1, W2, a2_src, a2_dst, b2,
         n_nodes, bpc, trace=False):
    x = np.asarray(x, dtype=np.float32)
    edge_index = np.asarray(edge_index)

    g = _prep_graph(edge_index, n_nodes, bpc)

    has_b1 = bool(np.abs(np.asarray(b1)).max() > 0)
    key = (4, n_nodes, bpc, has_b1, tuple(g["da"]), tuple(g["db"]))
    if key in _CACHE:
        nc = _CACHE[key]
    else:
        nc = _build_program(g, has_b1)
        _CACHE[key] = nc

    heads, hid = HEADS, HID
    W1 = np.asarray(W1, np.float32)
    W2 = np.asarray(W2, np.float32)
    w1s = np.stack([W1[:, h * hid:(h + 1) * hid] @ np.asarray(a1_src, np.float32)[h]
                    for h in range(heads)], axis=1)
    w1d = np.stack([W1[:, h * hid:(h + 1) * hid] @ np.asarray(a1_dst, np.float32)[h]
                    for h in range(heads)], axis=1)
    w1e_np = np.concatenate([W1, w1s, w1d], axis=1)
    w2s = (W2 @ np.asarray(a2_src, np.float32)[0])[:, None]
    w2d = (W2 @ np.asarray(a2_dst, np.float32)[0])[:, None]
    w2e_np = np.concatenate([W2, w2s, w2d], axis=1)

    # permuted xT (full), zero-padded; per-core stripes sliced below
    tbl_rows = g["tbl_rows"]
    stride = g["stride"]
    xT = np.zeros((IN_DIM, tbl_rows), dtype=np.float32)
    xT[:, g["pos"]] = x.T

    common = {
        "w1e": _bf16(w1e_np),
        "w2e": _bf16(w2e_np),
        "b1t": np.tile(np.asarray(b1, np.float32)[None, :], (128, 1)),
        "ident": np.eye(128, dtype=np.float32),
    }
    in_maps = []
    for c in range(CORES):
        in_maps.append({
            **common,
            "xTs": _bf16(xT[:, c * stride:(c + 1) * stride]),
            "idxa": g["idxa"][c], "idxb": g["idxb"][c],
        })

    res = run_bass_kernel_spmd(nc, in_maps, list(range(CORES)), trace=trace)

    out_full = np.empty((n_nodes, OUT_DIM), dtype=np.float32)
    npc = g["npc"]
    for c in range(CORES):
        out_full[g["nodes_of_core"][c]] = res.results[c]["out"][0:npc]
    out_full += np.asarray(b2, np.float32)[None, :]
    return out_full, res


def _bf16(a):
    import ml_dtypes
    return np.asarray(a, dtype=np.float32).astype(ml_dtypes.bfloat16)


def kernel(x, edge_index, W1, a1_src, a1_dst, b1, W2, a2_src, a2_dst, b2):
    out, _ = _run(x, edge_index, W1, a1_src, a1_dst, b1, W2, a2_src, a2_dst,
                  b2, n_nodes=N, bpc=49)
    return out


# revision 32
# speedup vs baseline: 1.0062x; 1.0062x over previous
"""Two-layer GAT (graph attention) kernel for 8 Trainium2 NeuronCores.

v2 strategy (destination-sharded edge parallelism, gather-prep optimized):
  * Nodes are degree-sorted and dealt round-robin to the 8 cores; each core
    aggregates messages for its own 6250 nodes only (no cross-core reduce).
  * Sharded front end: each core computes the layer-1 fat-row table for ITS
    stripe only (bf16, 512B rows: [h0|1|h1|1|h2|1|h3|1|a_src(4)|pad]), then an
    AllGather replicates the full table to every core's HBM.  The interleaved
    "ones" columns make the attention denominator fall out of the same
    slot-reduce as the messages (no separate denominator reduce).
  * Per-edge rows are fetched with dma_gather (SWDGE).  The Pool-engine
    descriptor-prep cost is linear in the static index count, so padding is
    minimized with OVERLAPPED index tables: table A = rows [0, 32768),
    table B = rows [17408, 50176) of the same tensor (int16 index range fits
    both exactly).  Edges whose source falls in the overlap are assigned to
    whichever side balances that destination's A/B slot counts.
  * A and B gathers of two consecutive destination blocks land in ONE SBUF
    tile (4 blocks worth for layer 2), halving per-gather fixed costs.
    Gathers rotate over 4 SWDGE queues (4 Q7 cpu pairs, 4 descriptor rings).
  * Padding slots point at a dummy row whose alpha is -1e30 => exp() == 0.
  * Layer 2 repeats the scheme with 256B f32 rows [h2(32)|1|a2_src|pad].

The host side (pure numpy) permutes nodes, builds the padded gather index
lists, and un-permutes the result.
"""

import sys

sys.path.insert(0, "/opt/trn_rl_repo")

import numpy as np

import concourse.bacc as bacc
import concourse.bass as bass
import concourse.mybir as mybir
import concourse.tile as tile
from concourse.bass_utils import run_bass_kernel_spmd

F32 = mybir.dt.float32
BF16 = mybir.dt.bfloat16
I16 = mybir.dt.int16
AL = mybir.AluOpType
ACT = mybir.ActivationFunctionType

CORES = 8
NEG_SLOPE = 0.2
NEG_BIG = -1.0e30

# problem constants (nn_GAT_35296041238878)
N = 50000
IN_DIM = 128
HID = 32
HEADS = 4
OUT_DIM = 32

# layer-1 fat row (bf16): [h0(32)|1|h1(32)|1|h2(32)|1|h3(32)|1|asrc(4)|pad] = 256
L1_ROW = 256
L1_USE = HEADS * (HID + 1)          # 132 (h+ones)
L1H = HEADS * HID                   # 128
W1N = L1H + 2 * HEADS               # 136 matmul cols [h|asrc|adst]
# layer-2 fat row (f32): [h2(32)|1|a2s|pad] = 64
L2_ROW = 64
L2_USE = OUT_DIM + 1                # 33
W2N = OUT_DIM + 2                   # 34 matmul cols [h2|a2s|a2d]

NQ = 4                              # SWDGE queues
L1_GRP = 2                          # dst blocks per gather, layer 1
L2_GRP = 4                          # dst blocks per gather, layer 2

_CACHE = {}

# ---------------------------------------------------------------------------
# Tile's DMASW lane round-robin is not SWDGE-queue-aware: a lane semaphore is
# locked to the queue of its first user, so rotating queue_num with the
# default assignment trips "locked to SWDGE queue" at schedule time.
# Partition the 8 lanes: queue q -> lanes [q*2, q*2+2).
import concourse.tile_sem_assignment as _tsa


def _queue_aware_assign_tick(self, inst):
    q = getattr(inst, "queue_num", None)
    if q is not None and isinstance(inst, _tsa.DMAInst) \
            and inst.engine == _tsa.mybir.EngineType.Pool:
        if not hasattr(self, "_q_lane_ctr"):
            self._q_lane_ctr = {}
        ctr = self._q_lane_ctr.get(q, 0)
        self._q_lane_ctr[q] = ctr + 1
        lanes = max(1, self.swdge_sem_count // NQ)
        self.next_sw_dma_idx = (q % NQ) * lanes + (ctr % lanes)
    return _tsa.TileClockTick._orig_assign_tick(self, inst)


if not hasattr(_tsa.TileClockTick, "_orig_assign_tick"):
    _tsa.TileClockTick._orig_assign_tick = _tsa.TileClockTick._assign_tick
    _tsa.TileClockTick._assign_tick = _queue_aware_assign_tick


# ----------------------------------------------------------------------------
# host-side graph preprocessing
# ----------------------------------------------------------------------------
def _prep_graph(edge_index, n_nodes, bpc):
    """Permute nodes, shard by destination, build padded gather index lists.

    Index tables overlap: A = rows [0, 32768), B = rows [BBASE, tbl_rows)
    with BBASE = tbl_rows - 32768.  Edges with src pos in the overlap are
    assigned to balance each node's A/B slot counts.
    """
    npc = n_nodes // CORES           # real nodes per core
    stride = bpc * 128               # table stripe per core (rows >= npc: dummy)
    tbl_rows = CORES * stride
    bbase = tbl_rows - 32768
    assert npc < stride and bbase >= 0 and tbl_rows - bbase == 32768
    a_dummy = npc                    # core-0 stripe dummy row, < 32768
    bd_core = next(c for c in range(CORES) if c * stride + npc >= bbase)
    b_dummy_local = bd_core * stride + npc - bbase
    assert 0 <= b_dummy_local < 32768

    src = np.concatenate([edge_index[0], np.arange(n_nodes)]).astype(np.int64)
    dst = np.concatenate([edge_index[1], np.arange(n_nodes)]).astype(np.int64)

    deg = np.bincount(dst, minlength=n_nodes)
    order = np.argsort(-deg, kind="stable")
    # rank r -> core r%8, local row r//8  (degree-balanced, within-core sorted)
    pos = np.empty(n_nodes, dtype=np.int64)
    ranks = np.arange(n_nodes)
    pos[order] = (ranks % CORES) * stride + ranks // CORES
    nodes_of_core = [order[c::CORES] for c in range(CORES)]

    dpos = pos[dst]
    e_core = dpos // stride
    ld = dpos % stride               # local dst row, < npc
    sp = pos[src]                    # source table position

    # ---- balanced A/B side assignment ----
    key = e_core * stride + ld       # destination node's table row
    fixedB = sp >= 32768
    flex = (sp >= bbase) & ~fixedB
    degn = np.bincount(key, minlength=tbl_rows)
    nA_fixed = np.bincount(key[sp < bbase], minlength=tbl_rows)
    nF = np.bincount(key[flex], minlength=tbl_rows)
    tgtA = np.minimum(np.maximum((degn + 1) // 2, nA_fixed), nA_fixed + nF)
    # rank of each flex edge within its key
    fidx = np.flatnonzero(flex)
    o = np.argsort(key[fidx], kind="stable")
    fs = fidx[o]
    ks = key[fs]
    change = np.r_[True, ks[1:] != ks[:-1]]
    starts = np.flatnonzero(change)
    gid = np.cumsum(change) - 1
    frank = np.arange(len(fs)) - starts[gid]
    sideB = fixedB.copy()
    sideB[fs] = frank >= (tgtA - nA_fixed)[ks]

    nA = np.bincount(key[~sideB], minlength=tbl_rows)
    nB = degn - nA

    def blockmax(x):
        return x.reshape(CORES, bpc, 128).max(axis=0).max(axis=1)

    da = np.maximum(blockmax(nA), 1)
    db = np.maximum(blockmax(nB), 1)
    offa = np.concatenate([[0], np.cumsum(da)])
    offb = np.concatenate([[0], np.cumsum(db)])

    idxa_list, idxb_list = [], []
    for c in range(CORES):
        m = e_core == c
        ldc, spc, sbc = ld[m], sp[m], sideB[m]
        o2 = np.lexsort((sbc, ldc))
        ldc, spc, sbc = ldc[o2], spc[o2], sbc[o2]
        keyc = ldc * 2 + sbc
        change = np.r_[True, keyc[1:] != keyc[:-1]]
        gid = np.cumsum(change) - 1
        starts = np.flatnonzero(change)
        jj = np.arange(len(ldc)) - starts[gid]
        bidx = ldc // 128
        d = ldc % 128
        flat_a = np.full(128 * offa[-1], a_dummy, dtype=np.int64)
        flat_b = np.full(128 * offb[-1], b_dummy_local, dtype=np.int64)
        ma = ~sbc
        flat_a[(offa[bidx[ma]] + jj[ma]) * 128 + d[ma]] = spc[ma]
        mb = sbc
        flat_b[(offb[bidx[mb]] + jj[mb]) * 128 + d[mb]] = spc[mb] - bbase
        assert flat_a.max() < 32768 and flat_b.max() < 32768
        # wrap per block: i -> [i%16, i//16], concat blocks along columns
        wa = np.concatenate(
            [flat_a[128 * offa[b]:128 * offa[b + 1]].reshape(-1, 16).T
             for b in range(bpc)], axis=1).astype(np.int16)
        wb = np.concatenate(
            [flat_b[128 * offb[b]:128 * offb[b + 1]].reshape(-1, 16).T
             for b in range(bpc)], axis=1).astype(np.int16)
        idxa_list.append(np.tile(wa, (8, 1)))
        idxb_list.append(np.tile(wb, (8, 1)))

    return dict(
        npc=npc, stride=stride, tbl_rows=tbl_rows, bbase=bbase, bpc=bpc,
        da=da.astype(int).tolist(), db=db.astype(int).tolist(),
        offa=offa.astype(int).tolist(), offb=offb.astype(int).tolist(),
        pos=pos, nodes_of_core=nodes_of_core,
        idxa=idxa_list, idxb=idxb_list,
    )


# ----------------------------------------------------------------------------
# device program
# ----------------------------------------------------------------------------
def _build_program(g, has_b1):
    bpc, stride, tbl_rows, bbase = g["bpc"], g["stride"], g["tbl_rows"], g["bbase"]
    da, db, offa, offb = g["da"], g["db"], g["offa"], g["offb"]
    npc = g["npc"]
    sa_cols = 8 * offa[-1]
    sb_cols = 8 * offb[-1]

    nc = bacc.Bacc("TRN2", target_bir_lowering=False, debug=False,
                   num_devices=CORES, num_swdge_queues=NQ)

    xTs = nc.dram_tensor("xTs", [128, stride], BF16, kind="ExternalInput")
    w1e = nc.dram_tensor("w1e", [128, W1N], BF16, kind="ExternalInput")
    w2e = nc.dram_tensor("w2e", [L1H, W2N], BF16, kind="ExternalInput")
    b1t = nc.dram_tensor("b1t", [128, L1H], F32, kind="ExternalInput")
    ident = nc.dram_tensor("ident", [128, 128], F32, kind="ExternalInput")
    idxa = nc.dram_tensor("idxa", [128, sa_cols], I16, kind="ExternalInput")
    idxb = nc.dram_tensor("idxb", [128, sb_cols], I16, kind="ExternalInput")

    cc1 = nc.dram_tensor("cc1", [stride, L1_ROW], BF16)
    tbl1 = nc.dram_tensor("tbl1", [tbl_rows, L1_ROW], BF16, addr_space="Shared")
    cc2 = nc.dram_tensor("cc2", [stride, L2_ROW], F32)
    tbl2 = nc.dram_tensor("tbl2", [tbl_rows, L2_ROW], F32, addr_space="Shared")
    out = nc.dram_tensor("out", [stride, OUT_DIM], F32, kind="ExternalOutput")

    with tile.TileContext(nc) as tc:
        with (
            tc.tile_pool(name="res", bufs=1) as res,
            tc.tile_pool(name="ps", bufs=2, space="PSUM") as psp,
            tc.tile_pool(name="sml", bufs=2) as sml,
        ):
            # ---- resident constants ----
            w1e_t = res.tile([128, W1N], BF16, tag="w1e")
            nc.sync.dma_start(w1e_t[:], w1e.ap())
            w2e_t = res.tile([L1H, W2N], BF16, tag="w2e")
            nc.sync.dma_start(w2e_t[:], w2e.ap())
            b1_t = res.tile([128, L1H], F32, tag="b1")
            nc.sync.dma_start(b1_t[:], b1t.ap())
            id_t = res.tile([128, 128], F32, tag="ident")
            nc.sync.dma_start(id_t[:], ident.ap())
            ia_t = res.tile([128, sa_cols], I16, tag="idxa")
            nc.sync.dma_start(ia_t[:], idxa.ap())
            ib_t = res.tile([128, sb_cols], I16, tag="idxb")
            nc.sync.dma_start(ib_t[:], idxb.ap())
            ad_own = res.tile([128, bpc * HEADS], F32, tag="adown")
            ad2_own = res.tile([128, bpc], F32, tag="ad2own")

            # dummy rows [npc, stride) of both cc tensors: alpha = -1e30
            pad_rows = stride - npc
            dmy1 = res.tile([pad_rows, L1_ROW], BF16, tag="dmy1")
            nc.vector.memset(dmy1[:], 0.0)
            nc.vector.memset(dmy1[:, L1_USE:L1_USE + HEADS], NEG_BIG)
            nc.sync.dma_start(cc1.ap()[npc:stride, :], dmy1[:])
            # l2 row layout: [h2(0:32) | a2s(32) | one(33) | pad]
            dmy2 = res.tile([pad_rows, L2_ROW], F32, tag="dmy2")
            nc.vector.memset(dmy2[:], 0.0)
            nc.vector.memset(dmy2[:, OUT_DIM:OUT_DIM + 1], NEG_BIG)
            nc.sync.dma_start(cc2.ap()[npc:stride, :], dmy2[:])

            # ---- front end: this core's stripe of the fat-row table ----
            fe_ctx = tc.tile_pool(name="fe", bufs=3)
            fe = fe_ctx.__enter__()
            FCH = 4                   # blocks per cc1 write
            # pre-zero the 3 rotating fat buffers once; pads stay zero
            for _ in range(3):
                f0 = fe.tile([128, FCH, L1_ROW], BF16, tag="fat")
                nc.vector.memset(f0[:].rearrange("p a b -> p (a b)"), 0.0)
            tbl1_v = tbl1.ap().rearrange("(c s) e -> c s e", c=CORES)
            CH_ROWS = 1536            # AG chunk rows (12 frontend blocks)
            ag1_fired = 0

            for t0 in range(0, bpc, FCH):
                tn = min(FCH, bpc - t0)
                fat = fe.tile([128, FCH, L1_ROW], BF16, tag="fat")
                for k in range(tn):
                    t = t0 + k
                    xt = fe.tile([128, 128], BF16, tag="xt")
                    nc.sync.dma_start(xt[:], xTs.ap()[:, 128 * t:128 * (t + 1)])
                    ps = psp.tile([128, W1N], F32, tag="feps")
                    nc.tensor.matmul(ps[:], xt[:], w1e_t[:], start=True, stop=True)
                    fk = fat[:, k, :]
                    f4 = fk[:, 0:L1_USE].rearrange("p (h c) -> p h c", h=HEADS)
                    nc.vector.tensor_copy(
                        f4[:, :, 0:HID],
                        ps[:, 0:L1H].rearrange("p (h c) -> p h c", h=HEADS))
                    nc.vector.memset(f4[:, :, HID:HID + 1], 1.0)
                    nc.vector.tensor_copy(
                        fk[:, L1_USE:L1_USE + HEADS], ps[:, L1H:L1H + HEADS])
                    nc.vector.tensor_copy(
                        ad_own[:, HEADS * t:HEADS * (t + 1)],
                        ps[:, L1H + HEADS:L1H + 2 * HEADS])
                nrows = min(128 * tn, npc - 128 * t0)
                if nrows == 128 * tn:
                    nc.sync.dma_start(
                        cc1.ap()[128 * t0:128 * t0 + nrows, :].rearrange(
                            "(t p) e -> p t e", p=128), fat[:, 0:tn, :])
                else:
                    for k in range(tn):
                        t = t0 + k
                        nr = min(128, npc - 128 * t)
                        if nr > 0:
                            nc.sync.dma_start(
                                cc1.ap()[128 * t:128 * t + nr, :],
                                fat[0:nr, k, :])

            fe_ctx.__exit__(None, None, None)
            tc.strict_bb_all_engine_barrier()
            nc.gpsimd.collective_compute(
                "AllGather", AL.bypass,
                replica_groups=[list(range(CORES))],
                ins=[cc1.ap().opt()], outs=[tbl1.ap().opt()])
            tc.strict_bb_all_engine_barrier()

            # ---- layer 1: per-block gathers, tree slot-reduce ----
            l1_gat_ctx = tc.tile_pool(name="gat1", bufs=3)
            gat = l1_gat_ctx.__enter__()
            l1_mid_ctx = tc.tile_pool(name="mid1", bufs=3)
            mid = l1_mid_ctx.__enter__()
            # persistent double-buffered l2fat with constant cols pre-set
            l2f_tiles = []
            for i in range(2):
                lf = res.tile([128, L2_ROW], F32, tag=f"l2f{i}")
                nc.vector.memset(lf[:, OUT_DIM + 2:L2_ROW], 0.0)
                nc.vector.memset(lf[:, OUT_DIM + 1:OUT_DIM + 2], 1.0)
                l2f_tiles.append(lf)
            tblA = tbl1.ap()[0:32768, :]
            tblB = tbl1.ap()[bbase:tbl_rows, :]
            tbl2_v = tbl2.ap().rearrange("(c s) e -> c s e", c=CORES)
            ag2_fired = 0

            def tree_reduce(m, D, W):
                """In-place pairwise slot reduce of m[:, 0:D, 0:W] -> m[:,0,:].

                All adds are on flat contiguous [128, k*W] slabs.
                """
                Dt = 1 << (D.bit_length() - 1)
                if Dt == D and D > 1:
                    Dt >>= 1
                if D > Dt:
                    k = D - Dt
                    nc.vector.tensor_tensor(
                        m[:, 0:k, :].rearrange("p a b -> p (a b)"),
                        m[:, 0:k, :].rearrange("p a b -> p (a b)"),
                        m[:, Dt:D, :].rearrange("p a b -> p (a b)"), AL.add)
                k = Dt >> 1
                while k >= 1:
                    nc.vector.tensor_tensor(
                        m[:, 0:k, :].rearrange("p a b -> p (a b)"),
                        m[:, 0:k, :].rearrange("p a b -> p (a b)"),
                        m[:, k:2 * k, :].rearrange("p a b -> p (a b)"), AL.add)
                    k >>= 1

            def split_gathers(out_tile, tblA_ap, tblB_ap, idx_a, idx_b,
                              b, row, q0):
                """4 gathers per block (A and B halves) on 4 distinct queues."""
                DA, DB = da[b], db[b]
                parts = []
                hA = DA // 2
                if hA >= 1:
                    parts.append((out_tile[:, 0:hA, :], tblA_ap,
                                  idx_a[:, 8 * offa[b]:8 * (offa[b] + hA)], hA))
                    parts.append((out_tile[:, hA:DA, :], tblA_ap,
                                  idx_a[:, 8 * (offa[b] + hA):8 * offa[b + 1]],
                                  DA - hA))
                else:
                    parts.append((out_tile[:, 0:DA, :], tblA_ap,
                                  idx_a[:, 8 * offa[b]:8 * offa[b + 1]], DA))
                hB = DB // 2
                if hB >= 1:
                    parts.append((out_tile[:, DA:DA + hB, :], tblB_ap,
                                  idx_b[:, 8 * offb[b]:8 * (offb[b] + hB)], hB))
                    parts.append((out_tile[:, DA + hB:DA + DB, :], tblB_ap,
                                  idx_b[:, 8 * (offb[b] + hB):8 * offb[b + 1]],
                                  DB - hB))
                else:
                    parts.append((out_tile[:, DA:DA + DB, :], tblB_ap,
                                  idx_b[:, 8 * offb[b]:8 * offb[b + 1]], DB))
                for i, (oap, tap, iap, dn) in enumerate(parts):
                    nc.gpsimd.dma_gather(
                        oap, tap, iap, 128 * dn, 128 * dn, row,
                        elem_step=row, single_packet=False,
                        queue_num=(q0 + i) % NQ)

            for b in range(bpc):
                DA, DB = da[b], db[b]
                D = DA + DB
                gt = gat.tile([128, D, L1_ROW], BF16, tag="g")
                split_gathers(gt, tblA, tblB, ia_t, ib_t, b, L1_ROW, b % NQ)

                adb = ad_own[:, HEADS * b:HEADS * (b + 1)]
                z = sml.tile([128, D, HEADS], F32, tag="z")
                nc.vector.tensor_tensor(
                    z[:, :, :], gt[:, :, L1_USE:L1_USE + HEADS],
                    adb.unsqueeze(1).broadcast_to([128, D, HEADS]), AL.add)
                z2 = sml.tile([128, D, HEADS], F32, tag="z2")
                nc.vector.scalar_tensor_tensor(
                    z2[:].rearrange("p a b -> p (a b)"),
                    z[:].rearrange("p a b -> p (a b)"), NEG_SLOPE,
                    z[:].rearrange("p a b -> p (a b)"),
                    op0=AL.mult, op1=AL.max)
                wb = sml.tile([128, D, HEADS], BF16, tag="wb")
                nc.scalar.activation(
                    wb[:].rearrange("p a b -> p (a b)"),
                    z2[:].rearrange("p a b -> p (a b)"), ACT.Exp)

                m = mid.tile([128, D, L1_USE], F32, tag="m")
                m4 = m[:, :, :].rearrange("p d (h c) -> p d h c", h=HEADS)
                nc.vector.tensor_tensor(
                    m4, gt[:, :, 0:L1_USE].rearrange(
                        "p d (h c) -> p d h c", h=HEADS),
                    wb[:, :, :].unsqueeze(3).broadcast_to(
                        [128, D, HEADS, HID + 1]), AL.mult)
                tree_reduce(m, D, L1_USE)
                r4 = m[:, 0, :].rearrange("p (h c) -> p h c", h=HEADS)

                rec = sml.tile([128, HEADS], F32, tag="rec")
                nc.vector.reciprocal(rec[:], r4[:, :, HID])
                o1 = sml.tile([128, L1H], F32, tag="o1")
                nc.vector.tensor_tensor(
                    o1[:].rearrange("p (h c) -> p h c", h=HEADS),
                    r4[:, :, 0:HID],
                    rec[:].unsqueeze(2).broadcast_to([128, HEADS, HID]),
                    AL.mult)
                if has_b1:
                    o1b = sml.tile([128, L1H], F32, tag="o1b")
                    nc.vector.tensor_tensor(o1b[:], o1[:], b1_t[:, :], AL.add)
                else:
                    o1b = o1
                # elu(x) = max(x, exp(min(x,0)) - 1);  e1n = -min(x,0) = relu(-x)
                e1n = sml.tile([128, L1H], F32, tag="e1n")
                nc.scalar.activation(e1n[:], o1b[:], ACT.Relu, scale=-1.0)
                e2 = sml.tile([128, L1H], F32, tag="e2")
                nc.scalar.activation(e2[:], e1n[:], ACT.Exp, scale=-1.0)
                elu = sml.tile([128, L1H], F32, tag="elu")
                nc.vector.scalar_tensor_tensor(
                    elu[:], e2[:], -1.0, o1b[:], op0=AL.add, op1=AL.max)
                # h2' = elu^T @ W2ext
                tp = psp.tile([128, 128], F32, tag="tp")
                nc.tensor.transpose(tp[:], elu[:], id_t[:])
                eluT = sml.tile([128, 128], BF16, tag="eluT")
                nc.scalar.activation(eluT[:], tp[:], ACT.Copy)
                h2p = psp.tile([128, W2N], F32, tag="h2p")
                nc.tensor.matmul(h2p[:], eluT[:], w2e_t[:],
                                 start=True, stop=True)
                l2fat = l2f_tiles[b % 2]
                # l2fat row: [h2(0:32) | a2s(32) | one(33) | pad]
                nc.scalar.activation(
                    l2fat[:, 0:OUT_DIM + 1], h2p[:, 0:OUT_DIM + 1], ACT.Copy)
                nc.scalar.activation(
                    ad2_own[:, b:b + 1], h2p[:, OUT_DIM + 1:OUT_DIM + 2],
                    ACT.Copy)
                nrows = min(128, npc - 128 * b)
                nc.sync.dma_start(
                    cc2.ap()[128 * b:128 * b + nrows, :], l2fat[0:nrows, :])

            l1_mid_ctx.__exit__(None, None, None)
            l1_gat_ctx.__exit__(None, None, None)
            tc.strict_bb_all_engine_barrier()
            nc.gpsimd.collective_compute(
                "AllGather", AL.bypass,
                replica_groups=[list(range(CORES))],
                ins=[cc2.ap().opt()], outs=[tbl2.ap().opt()])
            tc.strict_bb_all_engine_barrier()

            # ---- layer 2: per-block gathers, tree slot-reduce ----
            l2_gat_ctx = tc.tile_pool(name="gat2", bufs=4)
            gat = l2_gat_ctx.__enter__()
            l2_mid_ctx = tc.tile_pool(name="mid2", bufs=3)
            mid = l2_mid_ctx.__enter__()
            t2A = tbl2.ap()[0:32768, :]
            t2B = tbl2.ap()[bbase:tbl_rows, :]
            W2R = OUT_DIM + 2      # reduce width: [h2|a2s(junk)|one]
            for b in range(bpc):
                DA, DB = da[b], db[b]
                D = DA + DB
                g2 = gat.tile([128, D, L2_ROW], F32, tag="g2")
                split_gathers(g2, t2A, t2B, ia_t, ib_t, b, L2_ROW, b % NQ)

                # z = a2s[src] + a2d[dst] on the scalar engine (strided read)
                z = sml.tile([128, D], F32, tag="z2l")
                nc.scalar.activation(
                    z[:, :], g2[:, :, OUT_DIM], ACT.Identity,
                    bias=ad2_own[:, b:b + 1])
                z2 = sml.tile([128, D], F32, tag="z2l2")
                nc.vector.scalar_tensor_tensor(
                    z2[:, :], z[:, :], NEG_SLOPE, z[:, :],
                    op0=AL.mult, op1=AL.max)
                w2t = sml.tile([128, D], F32, tag="w2t")
                nc.scalar.activation(w2t[:, :], z2[:, :], ACT.Exp)

                m2 = mid.tile([128, D, W2R], F32, tag="m2")
                nc.vector.tensor_tensor(
                    m2[:, :, :], g2[:, :, 0:W2R],
                    w2t[:, :].unsqueeze(2).broadcast_to([128, D, W2R]),
                    AL.mult)
                tree_reduce(m2, D, W2R)
                r = m2[:, 0, :]

                rec = sml.tile([128, 1], F32, tag="rec2")
                nc.vector.reciprocal(rec[:], r[:, OUT_DIM + 1:OUT_DIM + 2])
                o2 = sml.tile([128, OUT_DIM], F32, tag="o2")
                nc.vector.tensor_scalar(
                    o2[:], r[:, 0:OUT_DIM], rec[:], None, op0=AL.mult)
                nrows = min(128, npc - 128 * b)
                nc.sync.dma_start(
                    out.ap()[128 * b:128 * b + nrows, :], o2[0:nrows, :])

            l2_mid_ctx.__exit__(None, None, None)
            l2_gat_ctx.__exit__(None, None, None)

    nc.compile()
    return nc


# ----------------------------------------------------------------------------
# weight prep + end-to-end run
# ----------------------------------------------------------------------------
def _run(x, edge_index, W1, a1_src, a1_dst, b1, W2, a2_src, a2_dst, b2,
         n_nodes, bpc, trace=False):
    x = np.asarray(x, dtype=np.float32)
    edge_index = np.asarray(edge_index)

    g = _prep_graph(edge_index, n_nodes, bpc)

    has_b1 = bool(np.abs(np.asarray(b1)).max() > 0)
    key = (4, n_nodes, bpc, has_b1, tuple(g["da"]), tuple(g["db"]))
    if key in _CACHE:
        nc = _CACHE[key]
    else:
        nc = _build_program(g, has_b1)
        _CACHE[key] = nc

    heads, hid = HEADS, HID
    W1 = np.asarray(W1, np.float32)
    W2 = np.asarray(W2, np.float32)
    w1s = np.stack([W1[:, h * hid:(h + 1) * hid] @ np.asarray(a1_src, np.float32)[h]
                    for h in range(heads)], axis=1)
    w1d = np.stack([W1[:, h * hid:(h + 1) * hid] @ np.asarray(a1_dst, np.float32)[h]
                    for h in range(heads)], axis=1)
    w1e_np = np.concatenate([W1, w1s, w1d], axis=1)
    w2s = (W2 @ np.asarray(a2_src, np.float32)[0])[:, None]
    w2d = (W2 @ np.asarray(a2_dst, np.float32)[0])[:, None]
    w2e_np = np.concatenate([W2, w2s, w2d], axis=1)

    # permuted xT (full), zero-padded; per-core stripes sliced below
    tbl_rows = g["tbl_rows"]
    stride = g["stride"]
    xT = np.zeros((IN_DIM, tbl_rows), dtype=np.float32)
    xT[:, g["pos"]] = x.T

    common = {
        "w1e": _bf16(w1e_np),
        "w2e": _bf16(w2e_np),
        "b1t": np.tile(np.asarray(b1, np.float32)[None, :], (128, 1)),
        "ident": np.eye(128, dtype=np.float32),
    }
    in_maps = []
    for c in range(CORES):
        in_maps.append({
            **common,
            "xTs": _bf16(xT[:, c * stride:(c + 1) * stride]),
            "idxa": g["idxa"][c], "idxb": g["idxb"][c],
        })

    res = run_bass_kernel_spmd(nc, in_maps, list(range(CORES)), trace=trace)

    out_full = np.empty((n_nodes, OUT_DIM), dtype=np.float32)
    npc = g["npc"]
    for c in range(CORES):
        out_full[g["nodes_of_core"][c]] = res.results[c]["out"][0:npc]
    out_full += np.asarray(b2, np.float32)[None, :]
    return out_full, res


def _bf16(a):
    import ml_dtypes
    return np.asarray(a, dtype=np.float32).astype(ml_dtypes.bfloat16)


def kernel(x, edge_index, W1, a1_src, a1_dst, b1, W2, a2_src, a2_dst, b2):
    out, _ = _run(x, edge_index, W1, a1_src, a1_dst, b1, W2, a2_src, a2_dst,
                  b2, n_nodes=N, bpc=49)
    return out
